# revision 1
# baseline (speedup 1.0000x reference)
"""Trainium2 Bass kernel for nn_DecayTGNMemoryModule (scatter_memory).

Strategy (node-parallel, per sharding hint):
  - The 200000-row memory table and last_update are sharded row-wise across
    8 NeuronCores (25000 nodes per core).
  - Events are deduplicated on host (reference semantics: for duplicate node
    ids only the LAST event matters, and every event computes from the
    ORIGINAL memory), then routed to the owning shard.  Routing/layout only —
    all arithmetic (MLP, decay, GRU, gather, scatter) runs on device.
  - Per core, on device:
      * bulk-copy its memory shard to the output (DRAM->DRAM DMA)
      * dma_gather the event rows from the shard (event-major layout)
      * feature-major fp32r matmuls for the message MLP and GRU gates
        (biases folded into per-partition ACT bias reads)
      * decay scaling + GRU elementwise on DVE/ACT
      * PE transposes between event-major and feature-major layouts
      * one dma_scatter_add of (h_new - h_raw) onto the copied table
        (exact row replacement, since ids are unique after dedup)
  - Host concatenates the 8 output shards.
"""

import numpy as np

import concourse.bacc as bacc
import concourse.bass as bass
import concourse.mybir as mybir
import concourse.tile as tile
from concourse.bass_utils import run_bass_kernel_spmd
from concourse.masks import make_identity

NUM_NODES = 200000
MEM_DIM = 128
MSG_DIM = 172
DECAY = 0.1
N_CORES = 8
SHARD = NUM_NODES // N_CORES  # 25000
DUMMY_ROWS = 8  # scatter target for padding events
BLK = 512  # events per pipeline block
COPY_CHUNKS = 8
GS_CHUNK = 1024  # events per dma_gather/dma_scatter_add call (SWDGE ring limit)

F32 = mybir.dt.float32
F32R = mybir.dt.float32r
I16 = mybir.dt.int16
AF = mybir.ActivationFunctionType

_program_cache: dict = {}


def _build_program(C: int):
    """Build (and bacc-compile) the per-core Bass program for capacity C."""
    nb = C // 128
    nblk = C // BLK
    nc = bacc.Bacc(
        "TRN2",
        target_bir_lowering=False,
        debug=False,
        enable_asserts=True,
        num_devices=N_CORES,
    )

    mem = nc.dram_tensor("mem", [SHARD, MEM_DIM], F32, kind="ExternalInput")
    msgta = nc.dram_tensor("msgta", [128, C], F32R, kind="ExternalInput")
    msgtb = nc.dram_tensor("msgtb", [MSG_DIM - 128, C], F32R, kind="ExternalInput")
    tsem = nc.dram_tensor("tsem", [128, nb], F32, kind="ExternalInput")
    luem = nc.dram_tensor("luem", [128, nb], F32, kind="ExternalInput")
    gidx = nc.dram_tensor("gidx", [128, C // 16], I16, kind="ExternalInput")
    sidx = nc.dram_tensor("sidx", [128, C // 16], I16, kind="ExternalInput")
    w1ta = nc.dram_tensor("w1ta", [128, 128], F32R, kind="ExternalInput")
    w1tb = nc.dram_tensor("w1tb", [MSG_DIM - 128, 128], F32R, kind="ExternalInput")
    w2t = nc.dram_tensor("w2t", [128, 128], F32R, kind="ExternalInput")
    wiht = nc.dram_tensor("wiht", [128, 3 * 128], F32R, kind="ExternalInput")
    whht = nc.dram_tensor("whht", [128, 3 * 128], F32R, kind="ExternalInput")
    biases = nc.dram_tensor("biases", [128, 6], F32, kind="ExternalInput")
    out = nc.dram_tensor(
        "out", [SHARD + DUMMY_ROWS, MEM_DIM], F32, kind="ExternalOutput"
    )

    r32 = lambda ap: ap.bitcast(F32R)

    with tile.TileContext(nc) as tc:
        with (
            tc.tile_pool(name="const", bufs=1) as cp,
            tc.tile_pool(name="big", bufs=1) as bp,
            tc.tile_pool(name="msg", bufs=3) as mp,
            tc.tile_pool(name="wk", bufs=2) as wp,
            tc.tile_pool(name="ps", bufs=1, space="PSUM") as pp,
        ):
            def load(name, dram, shape, dtype=F32):
                t = cp.tile(shape, dtype, tag=name)
                nc.sync.dma_start(t[:], dram.ap())
                return t

            # identity (transposes need it) first: it runs on the Pool engine
            # which is saturated by gather/scatter desc-gen afterwards.
            ident = cp.tile([128, 128], F32, tag="ident")
            make_identity(nc, ident[:])

            # index tiles next, loaded via SWDGE so no cross-ring wait: the
            # gathers depend only on these and dominate the critical path.
            gidx_s = cp.tile([128, C // 16], I16, tag="gidx")
            nc.gpsimd.dma_start(gidx_s[:], gidx.ap())
            sidx_s = cp.tile([128, C // 16], I16, tag="sidx")
            nc.gpsimd.dma_start(sidx_s[:], sidx.ap())

            # gather all event rows: h_raw[r, q, :] = mem[idx(q*128+r)]
            # chunked: one instruction per GS_CHUNK idxs to fit the SWDGE
            # descriptor ring (a single 7K-row gather overflows it).
            h_raw = bp.tile([128, nb, 128], F32, tag="h_raw")
            for g0 in range(0, C, GS_CHUNK):
                gn = min(GS_CHUNK, C - g0)
                nc.gpsimd.dma_gather(
                    h_raw[:, g0 // 128 : (g0 + gn) // 128, :],
                    mem.ap(),
                    gidx_s[:, g0 // 16 : (g0 + gn) // 16],
                    gn,
                    gn,
                    MEM_DIM,
                )

            w1ta_s = load("w1ta", w1ta, [128, 128], F32R)
            w1tb_s = load("w1tb", w1tb, [MSG_DIM - 128, 128], F32R)
            w2t_s = load("w2t", w2t, [128, 128], F32R)
            wiht_s = load("wiht", wiht, [128, 384], F32R)
            whht_s = load("whht", whht, [128, 384], F32R)
            bias_s = load("biases", biases, [128, 6])
            ts_s = load("tsem", tsem, [128, nb])
            lu_s = load("luem", luem, [128, nb])

            bb = lambda col: bias_s[:, col : col + 1]

            # score = exp(-DECAY * max(ts - lu, 0)), event-major [128, nb]
            dt0 = cp.tile([128, nb], F32, tag="dt0")
            nc.vector.tensor_sub(dt0[:], ts_s[:], lu_s[:])
            dt1 = cp.tile([128, nb], F32, tag="dt1")
            nc.scalar.activation(dt1[:], dt0[:], AF.Relu)
            score = cp.tile([128, nb], F32, tag="score")
            nc.scalar.activation(score[:], dt1[:], AF.Exp, scale=-DECAY)

            # bulk copy mem shard -> out (rows that receive no event keep it).
            # Issued on the ACT HWDGE ring (not the SP ring carrying the
            # per-block message loads), interleaved into the block loop so the
            # 12.8MB doesn't monopolize SDMA while the gathers drain.
            def emit_copy_chunk(k):
                rows = SHARD // COPY_CHUNKS
                r0 = k * rows
                r1 = SHARD if k == COPY_CHUNKS - 1 else r0 + rows
                nc.scalar.dma_start(out[r0:r1, :], mem[r0:r1, :])

            delta = bp.tile([128, nb, 128], F32, tag="delta")

            for b in range(nblk):
                sl = slice(b * BLK, (b + 1) * BLK)
                q0 = b * (BLK // 128)
                nq = BLK // 128

                ma = mp.tile([128, BLK], F32R, tag="ma")
                nc.sync.dma_start(ma[:], msgta[:, sl])
                mb_ = mp.tile([MSG_DIM - 128, BLK], F32R, tag="mb")
                nc.sync.dma_start(mb_[:], msgtb[:, sl])

                # x1 = relu(W1 @ msg + b1)   (feature-major [128f, BLK])
                px1 = pp.tile([128, BLK], F32, tag="px1")
                nc.tensor.matmul(
                    px1[:], lhsT=w1ta_s[:], rhs=ma[:], start=True, stop=False
                )
                nc.tensor.matmul(
                    px1[:], lhsT=w1tb_s[:], rhs=mb_[:], start=False, stop=True
                )
                x1 = wp.tile([128, BLK], F32R, tag="x1")
                nc.scalar.activation(x1[:], px1[:], AF.Relu, bias=bb(0))

                # pm = W2 @ x1 + b2
                ppm = pp.tile([128, BLK], F32, tag="ppm")
                nc.tensor.matmul(
                    ppm[:], lhsT=w2t_s[:], rhs=x1[:], start=True, stop=True
                )
                pm = wp.tile([128, BLK], F32R, tag="pm")
                nc.scalar.activation(pm[:], ppm[:], AF.Identity, bias=bb(1))
                if b < COPY_CHUNKS:
                    emit_copy_chunk(b)

                # h_scaled (event-major) = h_raw * score
                hs = wp.tile([128, nq, 128], F32, tag="hs")
                nc.vector.tensor_mul(
                    hs[:],
                    h_raw[:, q0 : q0 + nq, :],
                    score[:, q0 : q0 + nq, None].to_broadcast([128, nq, 128]),
                )

                # transpose h_scaled to feature-major
                pht = pp.tile([128, BLK], F32, tag="pht")
                for j in range(nq):
                    nc.tensor.transpose(
                        pht[:, j * 128 : (j + 1) * 128], hs[:, j, :], ident[:]
                    )
                hf = wp.tile([128, BLK], F32R, tag="hf")
                nc.scalar.activation(hf[:], pht[:], AF.Copy)

                # gates: r/z accumulate gx+gh in PSUM; n parts kept separate
                pr = pp.tile([128, BLK], F32, tag="pr")
                nc.tensor.matmul(
                    pr[:], lhsT=wiht_s[:, 0:128], rhs=pm[:],
                    start=True, stop=False,
                )
                nc.tensor.matmul(
                    pr[:], lhsT=whht_s[:, 0:128], rhs=hf[:],
                    start=False, stop=True,
                )
                pz = pp.tile([128, BLK], F32, tag="pz")
                nc.tensor.matmul(
                    pz[:], lhsT=wiht_s[:, 128:256], rhs=pm[:],
                    start=True, stop=False,
                )
                nc.tensor.matmul(
                    pz[:], lhsT=whht_s[:, 128:256], rhs=hf[:],
                    start=False, stop=True,
                )
                pgx = pp.tile([128, BLK], F32, tag="pgx")
                nc.tensor.matmul(
                    pgx[:], lhsT=wiht_s[:, 256:384], rhs=pm[:],
                    start=True, stop=True,
                )
                pgh = pp.tile([128, BLK], F32, tag="pgh")
                nc.tensor.matmul(
                    pgh[:], lhsT=whht_s[:, 256:384], rhs=hf[:],
                    start=True, stop=True,
                )

                r_t = wp.tile([128, BLK], F32, tag="r")
                nc.scalar.activation(r_t[:], pr[:], AF.Sigmoid, bias=bb(2))
                z_t = wp.tile([128, BLK], F32, tag="z")
                nc.scalar.activation(z_t[:], pz[:], AF.Sigmoid, bias=bb(3))

                # rg = (gh_n + b_hh_n) * r
                rg = wp.tile([128, BLK], F32, tag="rg")
                nc.vector.scalar_tensor_tensor(
                    rg[:], pgh[:], bb(5), r_t[:],
                    op0=mybir.AluOpType.add, op1=mybir.AluOpType.mult,
                )
                npre = wp.tile([128, BLK], F32, tag="npre")
                nc.vector.tensor_add(npre[:], rg[:], pgx[:])
                n_t = wp.tile([128, BLK], F32, tag="n")
                nc.scalar.activation(n_t[:], npre[:], AF.Tanh, bias=bb(4))

                # h_new = n + z * (h_scaled - n)   (feature-major)
                d_t = wp.tile([128, BLK], F32, tag="d")
                nc.vector.tensor_sub(d_t[:], hf[:].bitcast(F32), n_t[:])
                zd = wp.tile([128, BLK], F32, tag="zd")
                nc.vector.tensor_mul(zd[:], z_t[:], d_t[:])
                hn = wp.tile([128, BLK], F32, tag="hn")
                nc.vector.tensor_add(hn[:], n_t[:], zd[:])

                # transpose h_new back to event-major; delta = h_new - h_raw
                phn = pp.tile([128, BLK], F32, tag="phn")
                for j in range(nq):
                    nc.tensor.transpose(
                        phn[:, j * 128 : (j + 1) * 128],
                        hn[:, j * 128 : (j + 1) * 128],
                        ident[:],
                    )
                nc.vector.tensor_sub(
                    delta[:, q0 : q0 + nq, :],
                    phn[:].rearrange("p (a b) -> p a b", a=nq),
                    h_raw[:, q0 : q0 + nq, :],
                )

            # scatter: out[idx] += delta  (row replacement after dedup)
            for g0 in range(0, C, GS_CHUNK):
                gn = min(GS_CHUNK, C - g0)
                nc.gpsimd.dma_scatter_add(
                    out.ap(),
                    delta[:, g0 // 128 : (g0 + gn) // 128, :],
                    sidx_s[:, g0 // 16 : (g0 + gn) // 16],
                    gn,
                    gn,
                    MEM_DIM,
                )

    nc.compile()
    return nc


def _prepare(inputs):
    """Host-side routing/layout: dedupe events (last wins), shard by node."""
    node_ids = np.asarray(inputs["node_ids"])
    messages = np.asarray(inputs["messages"], dtype=np.float32)
    timestamps = np.asarray(inputs["timestamps"], dtype=np.float32)
    memory = np.asarray(inputs["memory"], dtype=np.float32)
    last_update = np.asarray(inputs["last_update"], dtype=np.float32)

    B = node_ids.shape[0]
    ids = np.clip(node_ids.astype(np.int64), 0, NUM_NODES - 1)
    uniq, rev_first = np.unique(ids[::-1], return_index=True)
    last_pos = B - 1 - rev_first  # position of last event per unique id
    bounds = np.searchsorted(uniq, np.arange(N_CORES + 1) * SHARD)
    cmax = int(np.diff(bounds).max())
    C = max(BLK, -(-cmax // BLK) * BLK)
    nb = C // 128
    assert C <= 16384, f"per-shard event capacity {C} too large for SBUF plan"

    w1t = np.ascontiguousarray(np.asarray(inputs["W1"], np.float32).T)  # [172,128]
    wconst = {
        "w1ta": np.ascontiguousarray(w1t[:128]),
        "w1tb": np.ascontiguousarray(w1t[128:]),
        "w2t": np.ascontiguousarray(np.asarray(inputs["W2"], np.float32).T),
        "wiht": np.ascontiguousarray(np.asarray(inputs["W_ih"], np.float32).T),
        "whht": np.ascontiguousarray(np.asarray(inputs["W_hh"], np.float32).T),
    }
    b_ih = np.asarray(inputs["b_ih"], np.float32)
    b_hh = np.asarray(inputs["b_hh"], np.float32)
    biases = np.zeros((128, 6), np.float32)
    biases[:, 0] = np.asarray(inputs["b1"], np.float32)
    biases[:, 1] = np.asarray(inputs["b2"], np.float32)
    biases[:, 2] = b_ih[0:128] + b_hh[0:128]
    biases[:, 3] = b_ih[128:256] + b_hh[128:256]
    biases[:, 4] = b_ih[256:384]
    biases[:, 5] = b_hh[256:384]
    wconst["biases"] = biases

    in_maps = []
    for c in range(N_CORES):
        lo, hi = int(bounds[c]), int(bounds[c + 1])
        n = hi - lo
        nid = uniq[lo:hi]
        pos = last_pos[lo:hi]
        lid = (nid - c * SHARD).astype(np.int16)

        mT = messages[pos].T  # [172, n]
        msgta = np.zeros((128, C), np.float32)
        msgta[:, :n] = mT[:128]
        msgtb = np.zeros((MSG_DIM - 128, C), np.float32)
        msgtb[:, :n] = mT[128:]

        ts = np.zeros(C, np.float32)
        ts[:n] = timestamps[pos]
        lu = np.zeros(C, np.float32)
        lu[:n] = last_update[nid]

        gi = np.zeros(C, np.int16)
        gi[:n] = lid
        si = np.full(C, SHARD, np.int16)
        si[:n] = lid

        in_maps.append(
            {
                "mem": np.ascontiguousarray(memory[c * SHARD : (c + 1) * SHARD]),
                "msgta": msgta,
                "msgtb": msgtb,
                "tsem": np.ascontiguousarray(ts.reshape(nb, 128).T),
                "luem": np.ascontiguousarray(lu.reshape(nb, 128).T),
                "gidx": np.ascontiguousarray(np.tile(gi.reshape(C // 16, 16).T, (8, 1))),
                "sidx": np.ascontiguousarray(np.tile(si.reshape(C // 16, 16).T, (8, 1))),
                **wconst,
            }
        )
    return C, in_maps


def run(inputs, trace=False, tmpdir=None):
    """Shard, run on 8 cores, reassemble.  Returns (output, BassKernelResults)."""
    C, in_maps = _prepare(inputs)
    if C not in _program_cache:
        _program_cache[C] = _build_program(C)
    nc = _program_cache[C]
    res = run_bass_kernel_spmd(
        nc, in_maps, core_ids=list(range(N_CORES)), trace=trace, tmpdir=tmpdir
    )
    out = np.concatenate(
        [res.results[c]["out"][:SHARD] for c in range(N_CORES)], axis=0
    )
    return out, res


def kernel(**inputs) -> np.ndarray:
    out, _ = run(inputs)
    return out



# revision 7
# speedup vs baseline: 1.4018x; 1.4018x over previous
"""Trainium2 Bass kernel for nn_DecayTGNMemoryModule (scatter_memory).

Strategy (node-parallel, per sharding hint):
  - The 200000-row memory table and last_update are sharded row-wise across
    8 NeuronCores (25000 nodes per core, padded to 25088).
  - Events are deduplicated on host (reference semantics: for duplicate node
    ids only the LAST event matters, and every event computes from the
    ORIGINAL memory), then routed to the owning shard.  Routing/layout only -
    all arithmetic (MLP, decay, GRU, gather, scatter) runs on device.
  - The shard's rows are split into 16 fixed ranges of 1568 rows (+8 dummy
    rows each in the device layout).  Events are grouped by range and each
    group is padded to a common per-range capacity NC[c] (same across cores,
    so one compiled program serves all 8 SPMD cores).  Padded slots gather
    from / scatter to their range's dummy row, whose contents are discarded
    on reassembly.
  - Per core, on device:
      * 16 dma_gathers (one per range, spread over 4 SWDGE queues so their
        descriptor generation runs concurrently on different gpsimd core
        pairs) pull the event rows from DRAM.
      * bulk-copy mem -> out in 16 range-aligned chunks (ACT-ring D2D DMA),
        emitted early so the ring streams at full rate.
      * feature-major fp32r matmuls for the message MLP and GRU gates
        (biases folded into per-partition ACT bias reads), decay scaling +
        GRU elementwise on DVE/ACT, PE transposes between layouts.
      * 16 dma_scatter_adds of (h_new - h_raw), each onto a SLICED out range
        with range-relative indices: disjoint slices mean no WAW serialization
        between scatter chunks, and each chunk only waits for its own copy
        chunk - scatters pipeline with the copy and compute.
  - Host concatenates the 8 output shards.
"""

import numpy as np

import concourse.bacc as bacc
import concourse.bass as bass
import concourse.mybir as mybir
import concourse.tile as tile
from concourse.bass_utils import run_bass_kernel_spmd
from concourse.masks import make_identity

NUM_NODES = 200000
MEM_DIM = 128
MSG_DIM = 172
DECAY = 0.1
N_CORES = 8
SHARD = NUM_NODES // N_CORES  # 25000
NRANGE = 16                   # scatter/copy row ranges
RWR = 1568                    # real rows per range (16 * 1568 = 25088 >= 25000)
RW = RWR + 8                  # device range width incl. 8 dummy rows
NTAB = NRANGE * RW            # 25216 spread shard rows
DUMMY = RWR                   # range-relative dummy row (scatter pad target)
BLK = 512                     # events per pipeline block
GS_MAX = 1024                 # max idxs per SWDGE instruction (ring limit)
NQ = 4                        # SWDGE queues

F32 = mybir.dt.float32
F32R = mybir.dt.float32r
I16 = mybir.dt.int16
AF = mybir.ActivationFunctionType

_program_cache: dict = {}


def _build_program(NCs: tuple):
    """Build (and bacc-compile) the per-core Bass program for capacities NCs."""
    C = sum(NCs)
    nb = C // 128
    nblk = C // BLK
    offs = np.concatenate([[0], np.cumsum(NCs)]).astype(int)

    nc = bacc.Bacc(
        "TRN2",
        target_bir_lowering=False,
        debug=False,
        enable_asserts=True,
        num_devices=N_CORES,
        num_swdge_queues=NQ,
    )

    mem = nc.dram_tensor("mem", [NTAB, MEM_DIM], F32, kind="ExternalInput")
    msgta = nc.dram_tensor("msgta", [128, C], F32R, kind="ExternalInput")
    msgtb = nc.dram_tensor("msgtb", [MSG_DIM - 128, C], F32R, kind="ExternalInput")
    tsem = nc.dram_tensor("tsem", [128, nb], F32, kind="ExternalInput")
    luem = nc.dram_tensor("luem", [128, nb], F32, kind="ExternalInput")
    gidx = nc.dram_tensor("gidx", [128, C // 16], I16, kind="ExternalInput")
    sidx = nc.dram_tensor("sidx", [128, C // 16], I16, kind="ExternalInput")
    w1ta = nc.dram_tensor("w1ta", [128, 128], F32R, kind="ExternalInput")
    w1tb = nc.dram_tensor("w1tb", [MSG_DIM - 128, 128], F32R, kind="ExternalInput")
    w2t = nc.dram_tensor("w2t", [128, 128], F32R, kind="ExternalInput")
    wiht = nc.dram_tensor("wiht", [128, 3 * 128], F32R, kind="ExternalInput")
    whht = nc.dram_tensor("whht", [128, 3 * 128], F32R, kind="ExternalInput")
    biases = nc.dram_tensor("biases", [128, 6], F32, kind="ExternalInput")
    out = nc.dram_tensor("out", [NTAB, MEM_DIM], F32, kind="ExternalOutput")

    with tile.TileContext(nc) as tc:
        with (
            tc.tile_pool(name="const", bufs=1) as cp,
            tc.tile_pool(name="big", bufs=1) as bp,
            tc.tile_pool(name="msg", bufs=3) as mp,
            tc.tile_pool(name="wk", bufs=2) as wp,
            tc.tile_pool(name="ps", bufs=1, space="PSUM") as pp,
        ):
            def load(name, dram, shape, dtype=F32):
                t = cp.tile(shape, dtype, tag=name)
                nc.sync.dma_start(t[:], dram.ap())
                return t

            # identity (transposes need it) first: Pool engine.
            ident = cp.tile([128, 128], F32, tag="ident")
            make_identity(nc, ident[:])

            # index tiles via SWDGE so no cross-ring wait.
            gidx_s = cp.tile([128, C // 16], I16, tag="gidx")
            nc.gpsimd.dma_start(gidx_s[:], gidx.ap())
            sidx_s = cp.tile([128, C // 16], I16, tag="sidx")
            nc.gpsimd.dma_start(sidx_s[:], sidx.ap())

            # 16 range gathers up front, spread over the 4 SWDGE queues so
            # desc-gen runs concurrently on different gpsimd core pairs.
            h_raw = bp.tile([128, nb, 128], F32, tag="h_raw")
            for c in range(NRANGE):
                for g0 in range(offs[c], offs[c + 1], GS_MAX):
                    gn = min(GS_MAX, offs[c + 1] - g0)
                    nc.gpsimd.dma_gather(
                        h_raw[:, g0 // 128 : (g0 + gn) // 128, :],
                        mem.ap(),
                        gidx_s[:, g0 // 16 : (g0 + gn) // 16],
                        gn,
                        gn,
                        MEM_DIM,
                        queue_num=c % NQ,
                    )

            w1ta_s = load("w1ta", w1ta, [128, 128], F32R)
            w1tb_s = load("w1tb", w1tb, [MSG_DIM - 128, 128], F32R)
            w2t_s = load("w2t", w2t, [128, 128], F32R)
            wiht_s = load("wiht", wiht, [128, 384], F32R)
            whht_s = load("whht", whht, [128, 384], F32R)
            bias_s = load("biases", biases, [128, 6])
            ts_s = load("tsem", tsem, [128, nb])
            lu_s = load("luem", luem, [128, nb])

            bb = lambda col: bias_s[:, col : col + 1]

            # score = exp(-DECAY * max(ts - lu, 0)), event-major [128, nb]
            dt0 = cp.tile([128, nb], F32, tag="dt0")
            nc.vector.tensor_sub(dt0[:], ts_s[:], lu_s[:])
            dt1 = cp.tile([128, nb], F32, tag="dt1")
            nc.scalar.activation(dt1[:], dt0[:], AF.Relu)
            score = cp.tile([128, nb], F32, tag="score")
            nc.scalar.activation(score[:], dt1[:], AF.Exp, scale=-DECAY)

            # bulk copy mem -> out, 16 range-aligned chunks on the ACT ring.
            def emit_copy_chunk(k):
                r0, r1 = k * RW, (k + 1) * RW
                nc.scalar.dma_start(out[r0:r1, :], mem[r0:r1, :])

            # scatter chunk c: delta rows of range c onto out[r0:r1] with
            # range-relative idxs; disjoint slices -> chunks don't serialize.
            def emit_scatter_chunk(c):
                r0, r1 = c * RW, (c + 1) * RW
                for g0 in range(offs[c], offs[c + 1], GS_MAX):
                    gn = min(GS_MAX, offs[c + 1] - g0)
                    nc.gpsimd.dma_scatter_add(
                        out[r0:r1, :],
                        delta[:, g0 // 128 : (g0 + gn) // 128, :],
                        sidx_s[:, g0 // 16 : (g0 + gn) // 16],
                        gn,
                        gn,
                        MEM_DIM,
                        queue_num=c % NQ,
                    )

            delta = bp.tile([128, nb, 128], F32, tag="delta")
            next_sc = 0

            for b in range(nblk):
                sl = slice(b * BLK, (b + 1) * BLK)
                q0 = b * (BLK // 128)
                nq_ = BLK // 128

                ma = mp.tile([128, BLK], F32R, tag="ma")
                nc.sync.dma_start(ma[:], msgta[:, sl])
                mb_ = mp.tile([MSG_DIM - 128, BLK], F32R, tag="mb")
                nc.sync.dma_start(mb_[:], msgtb[:, sl])

                # x1 = relu(W1 @ msg + b1)   (feature-major [128f, BLK])
                px1 = pp.tile([128, BLK], F32, tag="px1")
                nc.tensor.matmul(
                    px1[:], lhsT=w1ta_s[:], rhs=ma[:], start=True, stop=False
                )
                nc.tensor.matmul(
                    px1[:], lhsT=w1tb_s[:], rhs=mb_[:], start=False, stop=True
                )
                x1 = wp.tile([128, BLK], F32R, tag="x1")
                nc.scalar.activation(x1[:], px1[:], AF.Relu, bias=bb(0))

                # pm = W2 @ x1 + b2
                ppm = pp.tile([128, BLK], F32, tag="ppm")
                nc.tensor.matmul(
                    ppm[:], lhsT=w2t_s[:], rhs=x1[:], start=True, stop=True
                )
                pm = wp.tile([128, BLK], F32R, tag="pm")
                nc.scalar.activation(pm[:], ppm[:], AF.Identity, bias=bb(1))

                # front-load the copy ring: 2 chunks per block.
                for k in (2 * b, 2 * b + 1):
                    if k < NRANGE:
                        emit_copy_chunk(k)

                # h_scaled (event-major) = h_raw * score
                hs = wp.tile([128, nq_, 128], F32, tag="hs")
                nc.vector.tensor_mul(
                    hs[:],
                    h_raw[:, q0 : q0 + nq_, :],
                    score[:, q0 : q0 + nq_, None].to_broadcast([128, nq_, 128]),
                )

                # transpose h_scaled to feature-major
                pht = pp.tile([128, BLK], F32, tag="pht")
                for j in range(nq_):
                    nc.tensor.transpose(
                        pht[:, j * 128 : (j + 1) * 128], hs[:, j, :], ident[:]
                    )
                hf = wp.tile([128, BLK], F32R, tag="hf")
                nc.scalar.activation(hf[:], pht[:], AF.Copy)

                # gates: r/z accumulate gx+gh in PSUM; n parts kept separate
                pr = pp.tile([128, BLK], F32, tag="pr")
                nc.tensor.matmul(
                    pr[:], lhsT=wiht_s[:, 0:128], rhs=pm[:],
                    start=True, stop=False,
                )
                nc.tensor.matmul(
                    pr[:], lhsT=whht_s[:, 0:128], rhs=hf[:],
                    start=False, stop=True,
                )
                pz = pp.tile([128, BLK], F32, tag="pz")
                nc.tensor.matmul(
                    pz[:], lhsT=wiht_s[:, 128:256], rhs=pm[:],
                    start=True, stop=False,
                )
                nc.tensor.matmul(
                    pz[:], lhsT=whht_s[:, 128:256], rhs=hf[:],
                    start=False, stop=True,
                )
                pgx = pp.tile([128, BLK], F32, tag="pgx")
                nc.tensor.matmul(
                    pgx[:], lhsT=wiht_s[:, 256:384], rhs=pm[:],
                    start=True, stop=True,
                )
                pgh = pp.tile([128, BLK], F32, tag="pgh")
                nc.tensor.matmul(
                    pgh[:], lhsT=whht_s[:, 256:384], rhs=hf[:],
                    start=True, stop=True,
                )

                r_t = wp.tile([128, BLK], F32, tag="r")
                nc.scalar.activation(r_t[:], pr[:], AF.Sigmoid, bias=bb(2))
                z_t = wp.tile([128, BLK], F32, tag="z")
                nc.scalar.activation(z_t[:], pz[:], AF.Sigmoid, bias=bb(3))

                # rg = (gh_n + b_hh_n) * r
                rg = wp.tile([128, BLK], F32, tag="rg")
                nc.vector.scalar_tensor_tensor(
                    rg[:], pgh[:], bb(5), r_t[:],
                    op0=mybir.AluOpType.add, op1=mybir.AluOpType.mult,
                )
                npre = wp.tile([128, BLK], F32, tag="npre")
                nc.vector.tensor_add(npre[:], rg[:], pgx[:])
                n_t = wp.tile([128, BLK], F32, tag="n")
                nc.scalar.activation(n_t[:], npre[:], AF.Tanh, bias=bb(4))

                # h_new = n + z * (h_scaled - n)   (feature-major)
                d_t = wp.tile([128, BLK], F32, tag="d")
                nc.vector.tensor_sub(d_t[:], hf[:].bitcast(F32), n_t[:])
                zd = wp.tile([128, BLK], F32, tag="zd")
                nc.vector.tensor_mul(zd[:], z_t[:], d_t[:])
                hn = wp.tile([128, BLK], F32, tag="hn")
                nc.vector.tensor_add(hn[:], n_t[:], zd[:])

                # transpose h_new back to event-major; delta = h_new - h_raw
                phn = pp.tile([128, BLK], F32, tag="phn")
                for j in range(nq_):
                    nc.tensor.transpose(
                        phn[:, j * 128 : (j + 1) * 128],
                        hn[:, j * 128 : (j + 1) * 128],
                        ident[:],
                    )
                nc.vector.tensor_sub(
                    delta[:, q0 : q0 + nq_, :],
                    phn[:].rearrange("p (a b) -> p a b", a=nq_),
                    h_raw[:, q0 : q0 + nq_, :],
                )

                # emit scatters whose delta range is now fully computed
                while next_sc < NRANGE and offs[next_sc + 1] <= (b + 1) * BLK:
                    emit_scatter_chunk(next_sc)
                    next_sc += 1

            for k in range(2 * nblk, NRANGE):
                emit_copy_chunk(k)
            while next_sc < NRANGE:
                emit_scatter_chunk(next_sc)
                next_sc += 1

    nc.compile()
    return nc


def _prepare(inputs):
    """Host-side routing/layout: dedupe events (last wins), shard by node,
    group per 1568-row range with per-range padding shared across shards."""
    node_ids = np.asarray(inputs["node_ids"])
    messages = np.asarray(inputs["messages"], dtype=np.float32)
    timestamps = np.asarray(inputs["timestamps"], dtype=np.float32)
    memory = np.asarray(inputs["memory"], dtype=np.float32)
    last_update = np.asarray(inputs["last_update"], dtype=np.float32)

    B = node_ids.shape[0]
    ids = np.clip(node_ids.astype(np.int64), 0, NUM_NODES - 1)
    uniq, rev_first = np.unique(ids[::-1], return_index=True)
    last_pos = B - 1 - rev_first  # position of last event per unique id
    bounds = np.searchsorted(uniq, np.arange(N_CORES + 1) * SHARD)

    # per-core local ids and range counts
    per_core = []
    cnts = np.zeros((N_CORES, NRANGE), dtype=np.int64)
    for c in range(N_CORES):
        lo, hi = int(bounds[c]), int(bounds[c + 1])
        nid = (uniq[lo:hi] - c * SHARD).astype(np.int64)
        pos = last_pos[lo:hi]
        per_core.append((nid, pos))
        cnts[c] = np.bincount(nid // RWR, minlength=NRANGE)

    NCs = ((cnts.max(axis=0) + 127) // 128 * 128).astype(np.int64)
    NCs[NCs == 0] = 128
    extra = (-NCs.sum()) % BLK
    NCs[-1] += extra
    assert NCs.max() <= 2 * GS_MAX, f"range capacity {NCs.max()} too large"
    offs = np.concatenate([[0], np.cumsum(NCs)]).astype(np.int64)
    C = int(offs[-1])
    nb = C // 128

    w1t = np.ascontiguousarray(np.asarray(inputs["W1"], np.float32).T)  # [172,128]
    wconst = {
        "w1ta": np.ascontiguousarray(w1t[:128]),
        "w1tb": np.ascontiguousarray(w1t[128:]),
        "w2t": np.ascontiguousarray(np.asarray(inputs["W2"], np.float32).T),
        "wiht": np.ascontiguousarray(np.asarray(inputs["W_ih"], np.float32).T),
        "whht": np.ascontiguousarray(np.asarray(inputs["W_hh"], np.float32).T),
    }
    b_ih = np.asarray(inputs["b_ih"], np.float32)
    b_hh = np.asarray(inputs["b_hh"], np.float32)
    biases = np.zeros((128, 6), np.float32)
    biases[:, 0] = np.asarray(inputs["b1"], np.float32)
    biases[:, 1] = np.asarray(inputs["b2"], np.float32)
    biases[:, 2] = b_ih[0:128] + b_hh[0:128]
    biases[:, 3] = b_ih[128:256] + b_hh[128:256]
    biases[:, 4] = b_ih[256:384]
    biases[:, 5] = b_hh[256:384]
    wconst["biases"] = biases

    wrap16 = lambda a: np.ascontiguousarray(
        np.tile(a.reshape(-1, 16).T, (8, 1))
    )

    # per-position pad defaults: each padded slot targets its range's dummy row
    chunk_of_slot = np.repeat(np.arange(NRANGE), NCs)
    gi_pad = (RW * chunk_of_slot + DUMMY).astype(np.int16)

    in_maps = []
    for c in range(N_CORES):
        nid, pos = per_core[c]
        n = nid.shape[0]
        rng = nid // RWR
        starts = np.searchsorted(nid, RWR * np.arange(NRANGE))
        j_index = offs[rng] + (np.arange(n) - starts[rng])

        mT = np.zeros((MSG_DIM, C), np.float32)
        mT[:, j_index] = messages[pos].T

        ts = np.zeros(C, np.float32)
        ts[j_index] = timestamps[pos]
        lu = np.zeros(C, np.float32)
        lu[j_index] = last_update[nid + c * SHARD]

        gi = gi_pad.copy()
        gi[j_index] = (nid + 8 * rng).astype(np.int16)  # spread-layout row
        si = np.full(C, DUMMY, np.int16)
        si[j_index] = (nid - RWR * rng).astype(np.int16)

        memp = np.zeros((NTAB, MEM_DIM), np.float32)
        mem_shard = np.zeros((NRANGE * RWR, MEM_DIM), np.float32)
        mem_shard[:SHARD] = memory[c * SHARD : (c + 1) * SHARD]
        memp.reshape(NRANGE, RW, MEM_DIM)[:, :RWR] = mem_shard.reshape(
            NRANGE, RWR, MEM_DIM
        )

        in_maps.append(
            {
                "mem": memp,
                "msgta": np.ascontiguousarray(mT[:128]),
                "msgtb": np.ascontiguousarray(mT[128:]),
                "tsem": np.ascontiguousarray(ts.reshape(nb, 128).T),
                "luem": np.ascontiguousarray(lu.reshape(nb, 128).T),
                "gidx": wrap16(gi),
                "sidx": wrap16(si),
                **wconst,
            }
        )
    return tuple(int(x) for x in NCs), in_maps


def run(inputs, trace=False, tmpdir=None):
    """Shard, run on 8 cores, reassemble.  Returns (output, BassKernelResults)."""
    NCs, in_maps = _prepare(inputs)
    if NCs not in _program_cache:
        _program_cache[NCs] = _build_program(NCs)
    nc = _program_cache[NCs]
    res = run_bass_kernel_spmd(
        nc, in_maps, core_ids=list(range(N_CORES)), trace=trace, tmpdir=tmpdir
    )
    out = np.concatenate(
        [
            res.results[c]["out"]
            .reshape(NRANGE, RW, MEM_DIM)[:, :RWR]
            .reshape(NRANGE * RWR, MEM_DIM)[:SHARD]
            for c in range(N_CORES)
        ],
        axis=0,
    )
    return out, res


def kernel(**inputs) -> np.ndarray:
    out, _ = run(inputs)
    return out
